# revision 11
# baseline (speedup 1.0000x reference)
"""Trainium2 Bass kernel for the attention-pooling module (v2).

Reference math (B=32, N=2048, D=512, K=256):
    vIp   = vI @ Wi                                   [B,N,K]
    vQp   = vQ @ Wq + bq                              [B,K]
    ha    = leaky_relu(vIp + vQp[:,None,:], 0.01)     [B,N,K]
    scores= ha @ Wp[:,0] + bp                         [B,N]   (bp cancels in softmax)
    pi    = softmax(scores, -1)                       [B,N]
    out   = einsum("bn,bnk->bk", pi, vIp) + vQp       [B,K]

v2 strategy (8 cores, data-parallel over B, 4 batches/core). The kernel is
DMA-bound: vI streams twice in fp8 (vIT for the vIp matmuls, natural-layout
for the u matmuls) = 8.3 MiB/core at ~358 GB/s ~= 24.5 us. Everything else
is scheduled to hide under that stream:
  - vQp is computed on the HOST (tiny) and shipped both as [K-part, b]
    columns (ACT bias) and [1, K] rows (final add). Kills the on-device
    Wq matmul preamble and 0.5 MiB of weight DMA.
  - All 4 scores phases run first, then all 4 attention phases: the ACT
    table switches Lrelu->Exp exactly once (1.3 us per switch).
  - scores [1,512] PSUM tiles are copied to SBUF by the (otherwise idle)
    GpSimd engine, not the DVE.
  - The [1,N] -> [128,16] scoresT redistribution = one small SWDGE DMA
    (gpsimd queue, so it never waits behind the big HBM streams) + one PE
    transpose (~0.2 us) instead of a 1.2 us XBAR DMA-transpose.
  - vIp supertiles are 1024 wide (fewer, longer matmuls; LDWEIGHTS stays
    hidden under the previous matmul's column stream).
  - Weight DMAs are split so the first vIp matmul only waits for ~70 KB
    of fp8 weights + the first quarter of vit[0].
"""

import os
import sys

sys.path.insert(0, "/opt/trn_rl_repo")

import numpy as np
import ml_dtypes

from concourse import bass, bacc, tile, mybir
from concourse.bass_utils import run_bass_kernel_spmd

dt = mybir.dt
F32, BF16, FP8 = dt.float32, dt.bfloat16, dt.float8e4
AF = mybir.ActivationFunctionType
ALU = mybir.AluOpType

B, N, D, K = 32, 2048, 512, 256
NCORES = 8
BLOC = B // NCORES           # 4 batches per core
SUP = 512                    # scores-matmul tile (PSUM-bank limited)
WSUP = 1024                  # vIp supertile / ha width
DC = D // 128                # 4 d chunks
KC = K // 128                # 2 k chunks
NEG = 0.01


def build_nc():
    nc = bacc.Bacc("TRN2", target_bir_lowering=False, debug=False)

    vit_d = nc.dram_tensor("vit", [BLOC, 128, 2, 2, N], FP8, kind="ExternalInput")
    vnat_d = nc.dram_tensor("vnat", [BLOC, 128, N // 128, D], FP8, kind="ExternalInput")
    f8pk_d = nc.dram_tensor("f8pk", [128, 1056], FP8, kind="ExternalInput")
    pk16_d = nc.dram_tensor("pk16", [128, 1040], BF16, kind="ExternalInput")
    pk32_d = nc.dram_tensor("pk32", [128, 9], F32, kind="ExternalInput")
    vqpr_d = nc.dram_tensor("vqpr", [1, BLOC, K], F32, kind="ExternalInput")
    out = nc.dram_tensor("out", [BLOC, K], F32, kind="ExternalOutput")

    DEBUG = bool(int(os.environ.get("KERNEL_DEBUG", "0")))
    DBG_B = int(os.environ.get("KERNEL_DEBUG_B", "0"))
    if DEBUG:
        d_ecol = nc.dram_tensor("d_ecol", [128, 16], FP8, kind="ExternalOutput")
        d_z = nc.dram_tensor("d_z", [1, 1], F32, kind="ExternalOutput")
        d_fin = nc.dram_tensor("d_fin", [1, K], F32, kind="ExternalOutput")

    with tile.TileContext(nc) as tc:
        with (
            tc.tile_pool(name="const", bufs=1) as cpool,
            tc.tile_pool(name="stream", bufs=4) as spool,
            tc.tile_pool(name="work", bufs=3) as wpool,
            tc.tile_pool(name="pmm", bufs=2, space=bass.MemorySpace.PSUM) as pmm,
            tc.tile_pool(name="psm", bufs=4, space=bass.MemorySpace.PSUM) as psm,
        ):
            # ---- weights (split so compute starts early) ----
            f8pk_sb = cpool.tile([128, 1056], FP8, tag="f8pk")
            pk16_sb = cpool.tile([128, 1040], BF16, tag="pk16")
            pk32_sb = cpool.tile([128, 9], F32, tag="pk32")
            vqpr_sb = cpool.tile([1, BLOC, K], F32, tag="vqpr")

            vit_tiles, vnat_tiles = [], []
            for b in range(BLOC):
                vit_tiles.append(
                    spool.tile([128, 2, 2, N], FP8, tag="vit", name=f"vit{b}")
                )
                vnat_tiles.append(
                    spool.tile([128, N // 128, D], FP8, tag="vnat", name=f"vnat{b}")
                )

            # issue order == stream order on the sync HWDGE queue
            nc.sync.dma_start(out=f8pk_sb[:], in_=f8pk_d[:])
            nc.sync.dma_start(out=pk32_sb[:], in_=pk32_d[:])
            nc.sync.dma_start(
                out=vit_tiles[0][:, :, :, 0:512], in_=vit_d[0][:, :, :, 0:512]
            )
            nc.sync.dma_start(out=pk16_sb[:], in_=pk16_d[:])
            nc.sync.dma_start(out=vqpr_sb[:], in_=vqpr_d[:])
            nc.sync.dma_start(
                out=vit_tiles[0][:, :, :, 512:N], in_=vit_d[0][:, :, :, 512:N]
            )
            nc.sync.dma_start(out=vit_tiles[1][:], in_=vit_d[1])
            nc.sync.dma_start(out=vnat_tiles[0][:], in_=vnat_d[0])
            nc.sync.dma_start(out=vit_tiles[2][:], in_=vit_d[2])
            nc.sync.dma_start(out=vnat_tiles[1][:], in_=vnat_d[1])
            nc.sync.dma_start(out=vit_tiles[3][:], in_=vit_d[3])
            nc.sync.dma_start(out=vnat_tiles[2][:], in_=vnat_d[2])
            nc.sync.dma_start(out=vnat_tiles[3][:], in_=vnat_d[3])

            wi8_sb = f8pk_sb[:, 0:1024].rearrange("p (c i k) -> p c i k", c=2, i=2)
            wp8_sb = f8pk_sb[:, 1024:1056].rearrange("p (i j) -> p i j", i=2)
            wib_sb = pk16_sb[:, 0:1024].rearrange("p (c k) -> p c k", c=DC)
            idb16 = pk16_sb[:, 1024:1040]          # [128,16]; rows 0:16 = I16
            vqpt_sb = pk32_sb[:, 0:8].rearrange("p (c b) -> p c b", c=KC)
            onesc_sb = pk32_sb[:, 8:9]

            out_sb = cpool.tile([1, BLOC, K], F32, tag="outb")
            scols = [None] * BLOC

            def phase_scores(b):
                vit = vit_tiles[b]
                scrow = wpool.tile([1, N], BF16, tag="scrow")
                for sp in range(N // WSUP):          # two 1024-wide supertiles
                    n0 = sp * WSUP
                    ha = wpool.tile([128, KC, WSUP], FP8, tag="ha")
                    for kc in range(KC):
                        vp = pmm.tile([128, WSUP], F32, tag="vp")
                        for h in range(2):           # matmul out <= 1 PSUM bank
                            for cc in range(2):
                                nc.tensor.matmul(
                                    vp[:, h * SUP : (h + 1) * SUP],
                                    wi8_sb[:, cc, :, kc * 128 : (kc + 1) * 128],
                                    vit[:, cc, :, n0 + h * SUP : n0 + (h + 1) * SUP],
                                    perf_mode=mybir.MatmulPerfMode.DoubleRow,
                                    start=(cc == 0),
                                    stop=(cc == 1),
                                )
                        # Wi is host-scaled x16 into fp8 normal range; ACT
                        # de-scales for free: ha = lrelu(vp/16 + vqp)
                        # Prelu == leaky relu, but shares the `exp_and_others`
                        # ACT table with Exp -> zero table reloads when the
                        # attention phases interleave with scores phases
                        nc.scalar.activation(
                            ha[:, kc, :], vp[:], AF.Prelu,
                            bias=vqpt_sb[:, kc, b : b + 1], scale=1.0 / 16, alpha=NEG,
                        )
                    for h in range(2):
                        scp = psm.tile(
                            [1, SUP], F32, tag="small", name=f"scp{b}_{sp}_{h}"
                        )
                        nc.tensor.matmul(
                            scp[:], wp8_sb[:, :, 0:1],
                            ha[:, :, h * SUP : (h + 1) * SUP],
                            perf_mode=mybir.MatmulPerfMode.DoubleRow,
                            start=True, stop=True,
                        )
                        nc.vector.tensor_copy(
                            scrow[0:1, n0 + h * SUP : n0 + (h + 1) * SUP], scp[:]
                        )
                # redistribute [1,N] -> [16,128] on the empty gpsimd SWDGE
                # queue, then one PE transpose -> [128,16] scoresT
                s16 = wpool.tile([16, 128], BF16, tag="s16")
                nc.gpsimd.dma_start(
                    out=s16[:], in_=scrow[0:1, :].rearrange("o (t p) -> o t p", p=128)
                )
                scolp = psm.tile([128, 16], BF16, tag="small", name=f"scolp{b}")
                nc.tensor.transpose(scolp[:], s16[:], idb16[0:16, :])
                scol = cpool.tile([128, 16], BF16, tag=f"scol{b}")
                scols[b] = scol
                nc.vector.tensor_copy(scol[:], scolp[:])

            def phase_attn(b):
                vnat, scol = vnat_tiles[b], scols[b]
                # [128, 2, 16]: pair partner at +16B so the DoubleRow
                # lhsT AP satisfies the 16B-step ISA constraint
                e_col = wpool.tile([128, 2, 16], FP8, tag="ecol")
                zp = wpool.tile([128, 1], F32, tag="zp")
                # Wp is host-scaled x8 (fp8 range); exp de-scales for free
                nc.scalar.activation(
                    e_col[:].rearrange("p i j -> p j i")[:, 0:8, :],
                    scol[:].rearrange("p (j i) -> p j i", i=2),
                    AF.Exp, scale=1.0 / 8, accum_out=zp[:],
                )
                # Z = sum over partitions of zp, on the idle GpSimd engine
                # (keeps the PE in fp8-DoubleRow mode, no f32 matmul)
                z_sb = wpool.tile([128, 1], F32, tag="zsb")
                nc.gpsimd.partition_all_reduce(
                    z_sb[:], zp[:], channels=128, reduce_op=bass.bass_isa.ReduceOp.add
                )
                invz = wpool.tile([1, 1], F32, tag="invz")
                nc.vector.reciprocal(invz[:], z_sb[0:1, :])

                # u = e @ vI on the PE: 8 accumulating fp8 DoubleRow matmuls
                ups = psm.tile([1, D], F32, tag="small")
                NT = N // 128
                for t in range(0, NT, 2):
                    nc.tensor.matmul(
                        ups[:],
                        e_col[:, :, t // 2 : t // 2 + 1],  # pair stride 16B
                        vnat[:, t : t + 2, :],
                        perf_mode=mybir.MatmulPerfMode.DoubleRow,
                        start=(t == 0),
                        stop=(t == NT - 2),
                    )
                u_sb = wpool.tile([1, D], BF16, tag="usb")
                nc.vector.tensor_copy(u_sb[:], ups[:])
                utp = psm.tile([128, DC, 2], BF16, tag="small")
                for c in range(DC):
                    nc.tensor.transpose(
                        utp[:, c, 0:1],
                        u_sb[0:1, c * 128 : (c + 1) * 128],
                        idb16[0:1, 0:1],
                    )
                ut_sb = wpool.tile([128, DC], BF16, tag="utsb")
                nc.vector.tensor_copy(ut_sb[:], utp[:, :, 0])

                # att = u @ Wi   [1, K]
                atp = psm.tile([1, K], F32, tag="small")
                for c in range(DC):
                    nc.tensor.matmul(
                        atp[:], ut_sb[:, c : c + 1], wib_sb[:, c, :],
                        start=(c == 0), stop=(c == DC - 1),
                    )
                fin = wpool.tile([1, K], F32, tag="fin")
                nc.vector.tensor_scalar(fin[:], atp[:], invz[:], None, ALU.mult)
                nc.vector.tensor_tensor(
                    out_sb[:, b, :], fin[:], vqpr_sb[:, b, :], ALU.add
                )
                if DEBUG and b == DBG_B:
                    nc.sync.dma_start(out=d_ecol[:, 0:8], in_=e_col[:, 0, 0:8])
                    nc.sync.dma_start(out=d_z[:], in_=z_sb[:])
                    nc.sync.dma_start(out=d_fin[:], in_=fin[:])

            # software pipeline: attention(b) hides under scores(b+1);
            # Prelu and Exp share one ACT table so this is thrash-free
            for b in range(BLOC + 1):
                if b < BLOC:
                    phase_scores(b)
                if b >= 1:
                    phase_attn(b - 1)

            nc.sync.dma_start(out=out[:, :], in_=out_sb[0:1, :, :])

    nc.compile()
    return nc


_NC = None


def _get_nc():
    global _NC
    if _NC is None:
        _NC = build_nc()
    return _NC


def kernel(vI, vQ, Wi, Wq, bq, Wp, bp, **_unused):
    vI = np.asarray(vI, dtype=np.float32)
    vQ = np.asarray(vQ, dtype=np.float32)
    Wi = np.asarray(Wi, dtype=np.float32)
    Wq = np.asarray(Wq, dtype=np.float32)
    bq = np.asarray(bq, dtype=np.float32)
    Wp = np.asarray(Wp, dtype=np.float32)
    # bp shifts every score equally -> cancels in softmax; ignored.

    bf = ml_dtypes.bfloat16
    f8 = ml_dtypes.float8_e4m3
    # host-side: cast to fp8 and pre-transpose to [B, DC, 128, N]
    vi8 = vI.astype(f8)
    # DoubleRow layout: d = cc*256 + i*128 + p  ->  [B, p, cc, i, N]
    viT = np.ascontiguousarray(
        vi8.transpose(0, 2, 1).reshape(B, 2, 2, 128, N).transpose(0, 3, 1, 2, 4)
    )
    vnat = np.ascontiguousarray(
        vi8.reshape(B, N // 128, 128, D).transpose(0, 2, 1, 3)
    )

    # vQp on host (fp32, exact)
    vQp = vQ @ Wq + bq                                           # [B, K]

    wi_r = Wi.reshape(DC, 128, K).transpose(1, 0, 2)             # [128,DC,K]
    wi8_dr = np.ascontiguousarray(
        (Wi * 16.0).reshape(2, 2, 128, K).transpose(2, 0, 1, 3)
    ).reshape(128, 1024)                                          # [128,(cc i K)]
    wp_h = Wp[:, 0].reshape(KC, 128).T                           # [128,KC]
    wp_pad = np.zeros((128, 2, 16), np.float32)
    wp_pad[:, :, 0] = wp_h * 8.0
    f8pk = np.concatenate(
        [wi8_dr, wp_pad.reshape(128, 32)], axis=1
    ).astype(f8)                                                  # [128,1056]

    idb16 = np.zeros((128, 16), np.float32)
    idb16[0:16, 0:16] = np.eye(16)
    pk16 = np.concatenate(
        [wi_r.reshape(128, DC * K), idb16], axis=1
    ).astype(bf)                                                  # [128,1040]

    onesc = np.ones((128, 1), np.float32)

    def pk32_for(core):
        vqpc = vQp[core * BLOC : (core + 1) * BLOC]               # [BLOC, K]
        # vqpt[p, kc, b] = vQp[b, kc*128+p]
        vqpt = vqpc.T.reshape(KC, 128, BLOC).transpose(1, 0, 2)   # [128,KC,BLOC]
        return np.ascontiguousarray(
            np.concatenate([vqpt.reshape(128, KC * BLOC), onesc], axis=1)
        ).astype(np.float32)                                      # [128,9]

    in_maps = []
    for c in range(NCORES):
        in_maps.append(
            {
                "vit": viT[c * BLOC : (c + 1) * BLOC],
                "vnat": vnat[c * BLOC : (c + 1) * BLOC],
                "f8pk": f8pk,
                "pk16": pk16,
                "pk32": pk32_for(c),
                "vqpr": np.ascontiguousarray(
                    vQp[c * BLOC : (c + 1) * BLOC].reshape(1, BLOC, K)
                ),
            }
        )

    nc = _get_nc()
    res = run_bass_kernel_spmd(
        nc, in_maps, list(range(NCORES)),
        trace=bool(int(os.environ.get("KERNEL_TRACE", "0"))),
        tmpdir=globals().get("TRACE_TMPDIR"),
    )
    kernel.last_results = res
    return np.concatenate([res.results[c]["out"] for c in range(NCORES)], axis=0)
